# revision 1
# baseline (speedup 1.0000x reference)
# nn_DifferenceCost kernel for Trainium2 (Bass), 8-core SPMD.  v3
#
# out[b,s,y,x] = ||ref[b,:,y,x] - tgt[b,:,y+oy,x+ox]||_2, 0 out of bounds.
# s_val = nr2 + nt2 - 2*cross via TensorEngine: per 128-pixel block
# (16 rows x 8 cols), mm1 computes cross against the 24x16 target halo
# window (384 streamed fp16 columns, K=128); mm2 (K=2) accumulates
# -(nt2[n] + nr2[m])/2 so PSUM = -s_val/2.  A single ACT pass computes
# Sqrt(-2*PSUM) straight into the fp16 staging buffer (out-of-bounds
# entries produce garbage that the host masks to zero by geometry).
# SP-issued DMAs dump only the per-row-pair band windows (160 of 384
# columns) to DRAM while compute continues; the host performs the
# band->output gather (pure data movement).
#
# Sync-slot workaround: the target ISA encodes one semaphore wait per
# instruction, but Tile emits several on some (PSUM-WAR + RAW; the
# kernel-tail drain collects every DMA lane).  _legalize_waits hoists
# excess waits onto inserted same-engine NoOps, preserving the exact
# synchronization one wait at a time.
import sys

if "/opt/trn_rl_repo" not in sys.path:
    sys.path.insert(0, "/opt/trn_rl_repo")

import numpy as np

import concourse.bass as bass
import concourse.mybir as mybir
from concourse import tile

F16 = mybir.dt.float16
F32 = mybir.dt.float32

B, C, H, W = 4, 128, 96, 192
D = 4                    # max displacement
NS = 9                   # shifts per axis
S = NS * NS              # 81
NY = 48                  # output rows per core
GY, GX = NY + 2 * D, W + 2 * D   # 56 x 200 target halo grid
BRY, BRX = 16, 8         # ref block: 16 rows x 8 cols = 128 pixels
WRY, WRX = BRY + 2 * D, BRX + 2 * D  # 24 x 16 target window
NW = WRY * WRX           # 384 streamed columns per block
NSLAB = NY // BRY        # 3 slabs of 16 rows
NXB = W // BRX           # 24 x-blocks
NBLK = NSLAB * NXB       # 72 blocks per core
NPAIR = BRY // 2         # 8 row-pairs per block
PBW = 10 * WRX           # 160: band window per row-pair (rows 2pg..2pg+10)
SENTINEL = 60000.0       # fp16-safe filler for out-of-bounds nt2


def build_program(shrink: bool = True, reps: int = 1) -> bass.Bass:
    nc = bass.Bass()
    tgt_d = nc.declare_dram_parameter("tgt", [C, GY, GX], F16, isOutput=False)
    ref_d = nc.declare_dram_parameter("ref", [C, NBLK, 128], F16, isOutput=False)
    tm2_d = nc.declare_dram_parameter("tm2", [2, GY, GX], F16, isOutput=False)
    rr2_d = nc.declare_dram_parameter("rr2", [2, NBLK, 128], F16, isOutput=False)
    out_d = nc.declare_dram_parameter(
        "out", [NSLAB, NPAIR, 16, NXB, PBW], F16, isOutput=True)

    OSB_F = NBLK * NW

    with tile.TileContext(nc) as tc:
        with (
            tc.tile_pool(name="big", bufs=1) as big,
            tc.tile_pool(name="pa", bufs=6, space="PSUM") as pap,
        ):
            tgt_sb = big.tile([C, GY, GX], F16)
            ref_sb = big.tile([C, NBLK, 128], F16)
            tm2_sb = big.tile([2, GY, GX], F16)
            rr2_sb = big.tile([2, NBLK, 128], F16)
            osb = big.tile([C, OSB_F], F16)

            for rep in range(reps):
              # chunked loads so slab 0 compute starts before input lands
              row_chunks = [(0, WRY), (WRY, BRY), (WRY + BRY, BRY)]
              for lo, n in row_chunks:
                nc.sync.dma_start(tgt_sb[:, lo:lo + n, :], tgt_d[:, lo:lo + n, :])
              for s0 in range(NSLAB):
                bl, bh = s0 * NXB, (s0 + 1) * NXB
                nc.sync.dma_start(ref_sb[:, bl:bh, :], ref_d[:, bl:bh, :])
              nc.sync.dma_start(tm2_sb[:], tm2_d[:])
              nc.sync.dma_start(rr2_sb[:], rr2_d[:])

              for s0 in range(NSLAB):
                for xb in range(NXB):
                    blk = s0 * NXB + xb
                    pa = pap.tile([128, NW], F32)
                    tgt_sl = tgt_sb[:, s0 * BRY:s0 * BRY + WRY,
                                    xb * BRX:xb * BRX + WRX]
                    tm2_sl = tm2_sb[:, s0 * BRY:s0 * BRY + WRY,
                                    xb * BRX:xb * BRX + WRX]
                    # mm2 first: it takes the PSUM-slot WAR wait (its
                    # operands are long-resident), so mm1 carries only the
                    # tgt-chunk RAW -- every instruction stays at <=1 wait.
                    nc.tensor.matmul(pa[:], rr2_sb[:, blk, :], tm2_sl,
                                     start=True, stop=False)
                    nc.tensor.matmul(pa[:], ref_sb[:, blk, :], tgt_sl,
                                     start=False, stop=True)
                    # s_val = -2*PSUM; sqrt in the same ACT pass.  OOB
                    # entries are sqrt(negative) garbage, discarded by the
                    # host's band gather + geometric mask.
                    nc.scalar.activation(
                        osb[:, blk * NW:(blk + 1) * NW], pa[:],
                        mybir.ActivationFunctionType.Sqrt, scale=-2.0)
                # banded dumps via SP HWDGE: row-pair pg only ever needs
                # window columns [32pg, 32pg+160).
                for pg in range(NPAIR):
                    src = bass.AP(
                        osb.tensor,
                        (16 * pg) * OSB_F + (s0 * NXB) * NW + 32 * pg,
                        [[OSB_F, 16], [NW, NXB], [1, PBW]],
                    )
                    nc.sync.dma_start(out=out_d[s0, pg], in_=src)

    if shrink:
        _legalize_waits(nc)
    return nc


def _legalize_waits(nc) -> None:
    """The target ISA encodes at most ONE semaphore wait per instruction,
    but Tile emits instructions with several (PSUM-slot WAR + data RAW on
    hot ops; the kernel-tail drain collects every lane).  Hoist all but
    one wait of each such instruction onto freshly inserted same-engine
    NoOps placed immediately before it: the engine executes the NoOps'
    waits in program order, so the synchronization is preserved exactly,
    one wait per instruction."""
    for f in nc.m.functions:
        for b in f.blocks:
            il = b.instructions
            idx = 0
            while idx < len(il):
                ins = il[idx]
                si = ins.sync_info
                nw = len(si.on_wait) if si and si.on_wait else 0
                if nw > 1:
                    waits = list(si.on_wait)
                    for w in waits[:-1]:
                        nop = nc.engines[ins.engine].nop()
                        nop_ins = nop.ins if hasattr(nop, "ins") else nop
                        removed = False
                        for bb2 in f.blocks:
                            lst = bb2.instructions
                            if lst and lst[-1].name == nop_ins.name:
                                lst.pop()
                                removed = True
                                break
                        assert removed, "could not relocate wait NoOp"
                        nop_ins.sync_info = mybir.SyncInfo(
                            on_wait=[w], on_update=[])
                        il.insert(idx, nop_ins)
                        idx += 1
                    ins.sync_info = mybir.SyncInfo(
                        on_wait=[waits[-1]], on_update=si.on_update)
                idx += 1


def make_in_maps(reference_fm: np.ndarray, target_fm: np.ndarray):
    rh = reference_fm.astype(np.float16)
    th = target_fm.astype(np.float16)
    nr2 = (rh.astype(np.float32) ** 2).sum(axis=1)  # [B, H, W]
    nt2 = (th.astype(np.float32) ** 2).sum(axis=1)
    in_maps = []
    for c in range(8):
        b, half = c // 2, c % 2
        y0 = half * NY
        r_lo, r_hi = max(0, y0 - D), min(H, y0 + NY + D)
        g_lo = r_lo - (y0 - D)

        tgt_slab = np.zeros((C, GY, GX), np.float16)
        tgt_slab[:, g_lo:g_lo + (r_hi - r_lo), D:D + W] = th[b, :, r_lo:r_hi, :]

        tm2 = np.zeros((2, GY, GX), np.float32)
        tm2[0] = SENTINEL
        tm2[0, g_lo:g_lo + (r_hi - r_lo), D:D + W] = -0.5 * nt2[b, r_lo:r_hi, :]
        tm2[1] = 1.0

        # block-major ref: [C, blk, p] with blk = s0*24+xb, p = ry*8+rx
        ref_slab = rh[b, :, y0:y0 + NY, :].reshape(C, NSLAB, BRY, NXB, BRX)
        ref_slab = np.ascontiguousarray(
            ref_slab.transpose(0, 1, 3, 2, 4).reshape(C, NBLK, 128))

        nr_core = nr2[b, y0:y0 + NY, :]                    # [48, 192]
        rblk = nr_core.reshape(NSLAB, BRY, NXB, BRX)       # [s0, ry, xb, rx]
        rblk = rblk.transpose(0, 2, 1, 3).reshape(NBLK, 128)
        rr2 = np.stack([np.ones((NBLK, 128), np.float32), -0.5 * rblk])

        in_maps.append({
            "tgt": tgt_slab,
            "ref": ref_slab,
            "tm2": tm2.astype(np.float16),
            "rr2": rr2.astype(np.float16),
        })
    return in_maps


# ---- host-side band gather (pure data movement) ----
# out value for shift (soy, sox) at block pixel (ry, rx):
#   pair pg = ry//2, partition-in-pair pp = (ry%2)*8+rx,
#   band col = (ry+soy)*16 + (rx+sox) - 32*pg  (in [0, 160))
_RYg = np.arange(BRY)[None, :, None, None]
_RXg = np.arange(BRX)[None, None, None, :]
_SOYg = np.arange(NS)[:, None, None, None]
_SOXg = np.arange(NS)[None, None, :, None]
_PG = np.broadcast_to(_RYg // 2, (NS, BRY, NS, BRX))
_PP = np.broadcast_to((_RYg % 2) * 8 + _RXg, (NS, BRY, NS, BRX))
_COL = (_RYg + _SOYg) * WRX + (_RXg + _SOXg) - 32 * (_RYg // 2)


def assemble(results) -> np.ndarray:
    out = np.zeros((B, S, H, W), np.float32)
    for c in range(8):
        b, half = c // 2, c % 2
        o = np.asarray(results[c]["out"]).astype(np.float32)
        o = o.reshape(NSLAB, NPAIR, 16, NXB, PBW)
        # g[soy, ry, sox, rx, s0, xb] = o[s0, pg, pp, xb, col]
        g = o[:, _PG, _PP, :, _COL]
        # fancy-index result: [9,16,9,8, NSLAB, NXB]
        g = g.transpose(4, 0, 2, 1, 5, 3)        # [s0,soy,sox,ry,xb,rx]
        g = g.transpose(1, 2, 0, 3, 4, 5).reshape(S, NY, W)
        out[b, :, half * NY:half * NY + NY, :] = g
    # zero the out-of-bounds border of each shift (geometry only)
    for soy in range(NS):
        for sox in range(NS):
            s = soy * NS + sox
            oy, ox = soy - D, sox - D
            if oy < 0:
                out[:, s, :-oy, :] = 0.0
            elif oy > 0:
                out[:, s, H - oy:, :] = 0.0
            if ox < 0:
                out[:, s, :, :-ox] = 0.0
            elif ox > 0:
                out[:, s, :, W - ox:] = 0.0
    return out


_PROGRAM = None


def kernel(reference_fm: np.ndarray, target_fm: np.ndarray) -> np.ndarray:
    global _PROGRAM
    from concourse.bass_utils import run_bass_kernel_spmd

    reference_fm = np.asarray(reference_fm, dtype=np.float32)
    target_fm = np.asarray(target_fm, dtype=np.float32)
    if _PROGRAM is None:
        _PROGRAM = build_program()
    in_maps = make_in_maps(reference_fm, target_fm)
    res = run_bass_kernel_spmd(_PROGRAM, in_maps, core_ids=list(range(8)))
    return assemble(res.results)

